# revision 33
# baseline (speedup 1.0000x reference)
"""PASA group-softmax downsample kernel for 8 Trainium2 NeuronCores.

Reference computation (per reference.py):
  x (2, 64, 32, 32, 32) f32
  xp = reflect-pad x by 1 on d/h/w
  sigma = conv3d(xp, conv_w (54, 64, 3,3,3), stride 1, valid)   -> (2, 54, 32,32,32)
  sigma = batchnorm(sigma, batch stats over (n,d,h,w), gamma, beta)
  sigma = softmax(sigma, axis=1)
  out[n,g,cc,o] = sum_p patches[n,g,cc,p,o] * sigma[n,g*27+p,o]  (g=2 groups of 32 ch)
  return out[:, :, ::2, ::2, ::2]                                -> (2, 64, 16, 16, 16)

Sharding: 8 shards = (batch n in {0,1}) x (4 depth chunks of 8 planes).
Each core gets a padded depth slab with halo (10 planes of the padded
volume).

Two SPMD launches (a cross-core AllReduce measured ~80us +-70us of
launch-skew in this environment, vs a ~15us bare-launch floor, so the
432-byte BN-stat reduction is done on the host between launches):

Launch A (per core): stride-1 conv of the local slab as 27 shifted
  matmuls accumulated in PSUM (wl=0/1 tap pairs fused to K=128 via a
  +1-shifted copy of the slab in partitions 64..127; float32r at
  N>=256 runs 1 cycle/row). Per-plane BN partial sums + sum-of-squares
  and the strided-position sigma are extracted on DVE/ACT. Outputs:
  st (54, 2) partial stats, ssub (54, 1024) strided conv values.

Host: sum stats over cores -> mean/var -> a, b; E = exp(a*ssub + b);
  en = E / colsum(E)  (the group-softmax attention, pre-normalized).

Launch B (per core): for each strided depth plane (256 positions),
  replicate en rows across the 32 channels of each group with a
  partition-broadcast DMA, multiply against overlapping-window views
  of the fp32 slab (9 DVE multiplies of [64, 3x16x16]), reduce the 27
  taps with a contiguous halving tree, and DMA out (64, 4, 16, 16).
"""

import sys

sys.path.insert(0, "/opt/trn_rl_repo")

import numpy as np

import concourse.bacc as bacc
import concourse.mybir as mybir
from concourse import bass_utils, tile

N_CORES = 8
K = 3
GROUP = 2
STRIDE = 2
EPS = 1e-5

N, C, D, H, W = 2, 64, 32, 32, 32
COUT = GROUP * K * K * K  # 54
PD, PH, PW = D + 2, H + 2, W + 2  # 34, 34, 34
ZPLANES = 10  # 8 output planes + 2 halo planes of the padded volume
PLANE = PH * PW  # 1156
XLEN = ZPLANES * PLANE  # 11560
XBUF = XLEN + 8  # pad: junk-column reads of the last plane overrun slightly
DL = 8  # local output depth extent (stride-1)
SPOS = (DL // 2) * (H // 2) * (W // 2)  # 1024 strided positions per core
M_TOTAL = float(N * D * H * W)  # 65536 positions for BN stats

F32 = mybir.dt.float32
F32R = mybir.dt.float32r
F16 = mybir.dt.float16

# tap index p = di*9 + hj*3 + wl (matches reference im2col ordering)
# conv matmul units (tensor, di, hj, wl, K): 9 w-pairs (wl=0 with wl=1 via
# the +1-shifted upper half), 3 hj-pairs for wl=2 (hj=0 with hj=1 via a
# +34-shifted upper half), 3 singles (di, 2, 2) at K=64.
UNITS = (
    [("s1", di, hj, 0, 128) for di in range(K) for hj in range(K)]
    + [("s34", di, 0, 2, 128) for di in range(K)]
    + [("s1", di, 2, 2, 64) for di in range(K)]
)
NUNITS = len(UNITS)  # 15

_PROGRAM_CACHE = {}


def _build_weight_pack(conv_w: np.ndarray) -> np.ndarray:
    """Pack conv_w (54, 64, 3, 3, 3) into lhsT layout (128, 15*54)."""
    wpk = np.zeros((128, NUNITS * COUT), dtype=np.float32)
    for u, (tn, di, hj, wl, ku) in enumerate(UNITS):
        # lhsT[k, m]: k = input channel (row), m = output channel
        wpk[0:64, u * COUT : (u + 1) * COUT] = conv_w[:, :, di, hj, wl].T
        if ku == 128:
            if tn == "s1":  # upper half sees x shifted by +1 (next wl)
                wpk[64:128, u * COUT : (u + 1) * COUT] = conv_w[:, :, di, hj, wl + 1].T
            else:  # s34: upper half sees x shifted by +34 (next hj)
                wpk[64:128, u * COUT : (u + 1) * COUT] = conv_w[:, :, di, hj + 1, wl].T
    return wpk


def _win(t, parts, offset, dims):
    """Overlapping-window AP view of a [P, L] tile: free dims [(step, count), ...]."""
    v = t[0:parts, offset : offset + 1]
    for _ in range(len(dims) - 1):
        v = v.unsqueeze(1)
    w = v.copy()
    for i, (st, cnt) in enumerate(dims):
        w.ap[i + 1] = (st, cnt)
    return w


def _build_program_a():
    nc = bacc.Bacc(
        "TRN2", target_bir_lowering=False, debug=False, num_devices=N_CORES
    )
    xp2 = nc.dram_tensor("xp2", (128, XBUF), F32, kind="ExternalInput").ap()
    xq2 = nc.dram_tensor("xq2", (128, XBUF), F32, kind="ExternalInput").ap()
    wpk = nc.dram_tensor("wpk", (128, NUNITS * COUT), F32, kind="ExternalInput").ap()
    st = nc.dram_tensor("st", (COUT, 2), F32, kind="ExternalOutput").ap()
    ssub = nc.dram_tensor("ssub", (COUT, SPOS), F32, kind="ExternalOutput").ap()

    AX = mybir.AxisListType
    OP = mybir.AluOpType
    ACT = mybir.ActivationFunctionType

    with tile.TileContext(nc) as tc:
        with (
            tc.tile_pool(name="xin", bufs=1) as xin_pool,
            tc.tile_pool(name="consts", bufs=1) as const_pool,
            tc.tile_pool(name="stats", bufs=1) as stats_pool,
            tc.tile_pool(name="sq", bufs=2) as sq_pool,
        ):
            XPR = xin_pool.tile([128, XBUF], F32R)
            XS34 = xin_pool.tile([128, XBUF], F32R)
            WPK = const_pool.tile([128, NUNITS * COUT], F32)
            WPKR = const_pool.tile([128, NUNITS * COUT], F32R)

            nc.sync.dma_start(WPK[:], wpk[:])
            nc.vector.tensor_copy(WPKR[:], WPK[:])
            # input DMAs: stage per-plane, round f32 -> f32r into the slabs
            # (xp2: upper half +1-shifted; xq2: upper half +34-shifted)
            for z in range(ZPLANES):
                lo = z * PLANE
                hi = XBUF if z == ZPLANES - 1 else (z + 1) * PLANE
                STG = sq_pool.tile([128, PLANE + 8], F32, tag="stg")
                nc.sync.dma_start(STG[:, 0 : hi - lo], xp2[:, lo:hi])
                nc.vector.tensor_copy(XPR[:, lo:hi], STG[:, 0 : hi - lo])
                STG2 = sq_pool.tile([128, PLANE + 8], F32, tag="stg2")
                nc.scalar.dma_start(STG2[:, 0 : hi - lo], xq2[:, lo:hi])
                nc.vector.tensor_copy(XS34[:, lo:hi], STG2[:, 0 : hi - lo])

            SUMS = stats_pool.tile([COUT, DL], F32)
            SUMSQ = stats_pool.tile([COUT, DL], F32)
            SSUB = stats_pool.tile([COUT, SPOS], F32)

            # conv: per depth plane, flat span h*34+w (h<32, w<34; 2 junk
            # cols/row discarded at the stats/extraction step) in 3 groups
            SPAN = (PH - 2) * PW  # 1088
            GRPS = [(0, 364), (364, 364), (728, 360)]
            with tc.tile_pool(name="psum_conv", bufs=4, space="PSUM") as pconv:
                for d in range(DL):
                    SIG = sq_pool.tile([COUT, SPAN], F32, tag="sig")
                    for j0, ns in GRPS:
                        P = pconv.tile([COUT, 384], F32, tag="convps")
                        for u, (tn, di, hj, wl, ku) in enumerate(UNITS):
                            lhsT = WPKR[0:ku, u * COUT : (u + 1) * COUT]
                            base = (d + di) * PLANE + hj * PW + wl + j0
                            xt = XPR if tn == "s1" else XS34
                            rhs = xt[0:ku, base : base + ns]
                            nc.tensor.matmul(
                                P[:, 0:ns], lhsT, rhs,
                                start=(u == 0), stop=(u == NUNITS - 1),
                            )
                        nc.scalar.copy(SIG[:, j0 : j0 + ns], P[:, 0:ns])
                    # stats over the 32x32 valid positions of this plane
                    SIGv = SIG.rearrange("c (h w) -> c h w", h=H, w=PW)[:, :, 0:W]
                    nc.vector.tensor_reduce(
                        SUMS[:, d : d + 1], SIGv, axis=AX.XY, op=OP.add
                    )
                    SQT = sq_pool.tile([COUT, H * W], F32, tag="sqt")
                    SQTv = SQT.rearrange("c (h w) -> c h w", h=H, w=W)
                    nc.scalar.activation(
                        SQTv, SIGv, ACT.Square, accum_out=SUMSQ[:, d : d + 1]
                    )
                    if d % 2 == 0:
                        nc.scalar.copy(
                            SSUB[:, (d // 2) * 256 : (d // 2) * 256 + 256],
                            SIGv[:, 0:H:2, 0:W:2],
                        )

            ST = stats_pool.tile([COUT, 2], F32)
            nc.vector.tensor_reduce(ST[:, 0:1], SUMS[:], axis=AX.X, op=OP.add)
            nc.vector.tensor_reduce(ST[:, 1:2], SUMSQ[:], axis=AX.X, op=OP.add)
            nc.sync.dma_start(st[:], ST[:])
            nc.sync.dma_start(ssub[:], SSUB[:])
    nc.compile()
    return nc


def _build_program_b():
    nc = bacc.Bacc(
        "TRN2", target_bir_lowering=False, debug=False, num_devices=N_CORES
    )
    xpb = nc.dram_tensor("xpb", (64, XBUF), F32, kind="ExternalInput").ap()
    en = nc.dram_tensor("en", (4, 27 * GROUP * 256), F32, kind="ExternalInput").ap()
    out = nc.dram_tensor("out", (64, SPOS), F32, kind="ExternalOutput").ap()

    OP = mybir.AluOpType
    CH = 256  # one strided depth plane per chunk

    with tile.TileContext(nc) as tc:
        with (
            tc.tile_pool(name="xin", bufs=1) as xin_pool,
            tc.tile_pool(name="work", bufs=1) as work_pool,
        ):
            XPB = xin_pool.tile([64, XBUF], F32)
            # attention replication on the scalar HWDGE ring, slab loads on
            # the sync ring -> the two DMA streams don't queue behind each
            # other (HWDGE rings are FIFO per issuing engine)
            AREPS = [
                work_pool.tile([64, 27 * CH], F32, tag=f"arep{k}", name=f"AREP{k}")
                for k in range(4)
            ]
            # chunk 0's attention first, then slab planes interleaved with
            # the remaining attention blocks
            bounds = [0, 3 * PLANE, 5 * PLANE, 7 * PLANE, XBUF]
            for grp in range(2):
                nc.scalar.dma_start(
                    AREPS[0][grp * 32 : (grp + 1) * 32, :],
                    en[0:1, grp * 27 * CH : (grp + 1) * 27 * CH].partition_broadcast(32),
                )
            for k in range(1, 5):
                nc.sync.dma_start(
                    XPB[:, bounds[k - 1] : bounds[k]],
                    xpb[:, bounds[k - 1] : bounds[k]],
                )
                if k < 4:
                    for grp in range(2):
                        nc.scalar.dma_start(
                            AREPS[k][grp * 32 : (grp + 1) * 32, :],
                            en[
                                k : k + 1, grp * 27 * CH : (grp + 1) * 27 * CH
                            ].partition_broadcast(32),
                        )
            OUTC = work_pool.tile([64, SPOS], F32)
            for k in range(4):  # chunk = strided depth plane d4 = k
                AREP = AREPS[k]
                PRD = work_pool.tile([64, 27 * CH], F16, tag="prd", bufs=1)
                for di in range(K):
                    eng = nc.vector
                    for hj in range(K):
                        # patches [c, wl, h, w] = XPB[c, base + hj*34 + wl + h*68 + w*2]
                        xv = _win(
                            XPB, 64, (2 * k + di) * PLANE + hj * PW,
                            [(1, 3), (2 * PW, 16), (2, 16)],
                        )
                        sl = slice(
                            (di * 9 + hj * 3) * CH, (di * 9 + hj * 3 + 3) * CH
                        )
                        av = AREP[:, sl].rearrange(
                            "c (wl h w) -> c wl h w", wl=3, h=16, w=16
                        )
                        pvd = PRD[:, sl].rearrange(
                            "c (wl h w) -> c wl h w", wl=3, h=16, w=16
                        )
                        eng.tensor_tensor(pvd, xv, av, op=OP.mult)
                # reduce the 27 taps: halving tree on the (p, o) layout
                for lo, hi in [(11, 16), (8, 8), (4, 4), (2, 2)]:
                    nc.vector.tensor_add(
                        PRD[:, 0 : lo * CH],
                        PRD[:, 0 : lo * CH],
                        PRD[:, hi * CH : (hi + lo) * CH],
                    )
                nc.vector.tensor_add(
                    OUTC[:, k * CH : (k + 1) * CH],
                    PRD[:, 0:CH],
                    PRD[:, CH : 2 * CH],
                )
                nc.sync.dma_start(
                    out[:, k * CH : (k + 1) * CH], OUTC[:, k * CH : (k + 1) * CH]
                )
    nc.compile()
    return nc


def _prep_inputs_a(x, conv_w):
    xpad = np.pad(
        np.asarray(x, dtype=np.float32),
        ((0, 0), (0, 0), (1, 1), (1, 1), (1, 1)),
        mode="reflect",
    )
    wpk = _build_weight_pack(np.asarray(conv_w, dtype=np.float32))
    in_maps = []
    slabs = []
    for core in range(N_CORES):
        n, dc = core // 4, core % 4
        slab = xpad[n, :, 8 * dc : 8 * dc + ZPLANES].reshape(C, XLEN)
        xp2 = np.zeros((128, XBUF), dtype=np.float32)
        xp2[0:64, :XLEN] = slab
        xp2[64:128, : XLEN - 1] = slab[:, 1:]
        xq2 = np.zeros((128, XBUF), dtype=np.float32)
        xq2[0:64, :XLEN] = slab
        xq2[64:128, : XLEN - 34] = slab[:, 34:]
        in_maps.append({"xp2": xp2, "xq2": xq2, "wpk": wpk})
        slabs.append(np.ascontiguousarray(xp2[0:64]))
    return in_maps, slabs


def kernel(x, conv_w, bn_gamma, bn_beta):
    if "a" not in _PROGRAM_CACHE:
        _PROGRAM_CACHE["a"] = _build_program_a()
        _PROGRAM_CACHE["b"] = _build_program_b()
    nca, ncb = _PROGRAM_CACHE["a"], _PROGRAM_CACHE["b"]

    in_a, slabs = _prep_inputs_a(x, conv_w)
    res_a = bass_utils.run_bass_kernel_spmd(nca, in_a, core_ids=list(range(N_CORES)))

    # host: global BN stats (432 bytes per core), then normalized attention
    st = np.sum([r["st"] for r in res_a.results], axis=0, dtype=np.float64)
    mean = st[:, 0] / M_TOTAL
    var = st[:, 1] / M_TOTAL - mean * mean
    rstd = 1.0 / np.sqrt(var + EPS)
    a = np.asarray(bn_gamma, np.float64) * rstd
    b = np.asarray(bn_beta, np.float64) - mean * a

    in_b = []
    for core in range(N_CORES):
        ssub = res_a.results[core]["ssub"].astype(np.float64)
        e = np.exp(a[:, None] * ssub + b[:, None])
        en = (e / e.sum(axis=0, keepdims=True)).astype(np.float32)
        enr = np.ascontiguousarray(
            en.reshape(COUT, 4, 256).transpose(1, 0, 2)
        ).reshape(4, COUT * 256)
        in_b.append({"xpb": slabs[core], "en": enr})
    res_b = bass_utils.run_bass_kernel_spmd(ncb, in_b, core_ids=list(range(N_CORES)))

    full = np.empty((N, C, D // 2, H // 2, W // 2), dtype=np.float32)
    for core in range(N_CORES):
        n, dc = core // 4, core % 4
        full[n, :, 4 * dc : 4 * dc + 4] = res_b.results[core]["out"].reshape(
            64, 4, 16, 16
        )
    return full


# revision 35
# speedup vs baseline: 1.0520x; 1.0520x over previous
"""PASA group-softmax downsample kernel for 8 Trainium2 NeuronCores.

Reference computation (per reference.py):
  x (2, 64, 32, 32, 32) f32
  xp = reflect-pad x by 1 on d/h/w
  sigma = conv3d(xp, conv_w (54, 64, 3,3,3), stride 1, valid)   -> (2, 54, 32,32,32)
  sigma = batchnorm(sigma, batch stats over (n,d,h,w), gamma, beta)
  sigma = softmax(sigma, axis=1)
  out[n,g,cc,o] = sum_p patches[n,g,cc,p,o] * sigma[n,g*27+p,o]  (g=2 groups of 32 ch)
  return out[:, :, ::2, ::2, ::2]                                -> (2, 64, 16, 16, 16)

Sharding: 8 shards = (batch n in {0,1}) x (4 depth chunks of 8 planes).
Each core gets a padded depth slab with halo (10 planes of the padded
volume).

Two SPMD launches (a cross-core AllReduce measured ~80us +-70us of
launch-skew in this environment, vs a ~15us bare-launch floor, so the
432-byte BN-stat reduction is done on the host between launches):

Launch A (per core): stride-1 conv of the local slab as 27 shifted
  matmuls accumulated in PSUM (wl=0/1 tap pairs fused to K=128 via a
  +1-shifted copy of the slab in partitions 64..127; float32r at
  N>=256 runs 1 cycle/row). Per-plane BN partial sums + sum-of-squares
  and the strided-position sigma are extracted on DVE/ACT. Outputs:
  st (54, 2) partial stats, ssub (54, 1024) strided conv values.

Host: sum stats over cores -> mean/var -> a, b; E = exp(a*ssub + b);
  en = E / colsum(E)  (the group-softmax attention, pre-normalized).

Launch B (per core): for each strided depth plane (256 positions),
  replicate en rows across the 32 channels of each group with a
  partition-broadcast DMA, multiply against overlapping-window views
  of the fp32 slab (9 DVE multiplies of [64, 3x16x16] per chunk,
  products stored fp16 for the 2x 16-bit DVE rate), reduce the 27 taps
  with a contiguous halving tree in fp16 (final level re-expands to
  fp32), and DMA out (64, 4, 16, 16).
"""

import sys

sys.path.insert(0, "/opt/trn_rl_repo")

import numpy as np

import concourse.bacc as bacc
import concourse.mybir as mybir
from concourse import bass_utils, tile

N_CORES = 8
K = 3
GROUP = 2
STRIDE = 2
EPS = 1e-5

N, C, D, H, W = 2, 64, 32, 32, 32
COUT = GROUP * K * K * K  # 54
PD, PH, PW = D + 2, H + 2, W + 2  # 34, 34, 34
ZPLANES = 10  # 8 output planes + 2 halo planes of the padded volume
PLANE = PH * PW  # 1156
XLEN = ZPLANES * PLANE  # 11560
XBUF = XLEN + 8  # pad: junk-column reads of the last plane overrun slightly
DL = 8  # local output depth extent (stride-1)
SPOS = (DL // 2) * (H // 2) * (W // 2)  # 1024 strided positions per core
M_TOTAL = float(N * D * H * W)  # 65536 positions for BN stats

F32 = mybir.dt.float32
F32R = mybir.dt.float32r
F16 = mybir.dt.float16

# tap index p = di*9 + hj*3 + wl (matches reference im2col ordering)
# conv matmul units (tensor, di, hj, wl, K): 9 w-pairs (wl=0 with wl=1 via
# the +1-shifted upper half), 3 hj-pairs for wl=2 (hj=0 with hj=1 via a
# +34-shifted upper half), 3 singles (di, 2, 2) at K=64.
UNITS = (
    [("s1", di, hj, 0, 128) for di in range(K) for hj in range(K)]
    + [("s34", di, 0, 2, 128) for di in range(K)]
    + [("s1", di, 2, 2, 64) for di in range(K)]
)
NUNITS = len(UNITS)  # 15

_PROGRAM_CACHE = {}


def _build_weight_pack(conv_w: np.ndarray) -> np.ndarray:
    """Pack conv_w (54, 64, 3, 3, 3) into lhsT layout (128, 15*54)."""
    wpk = np.zeros((128, NUNITS * COUT), dtype=np.float32)
    for u, (tn, di, hj, wl, ku) in enumerate(UNITS):
        # lhsT[k, m]: k = input channel (row), m = output channel
        wpk[0:64, u * COUT : (u + 1) * COUT] = conv_w[:, :, di, hj, wl].T
        if ku == 128:
            if tn == "s1":  # upper half sees x shifted by +1 (next wl)
                wpk[64:128, u * COUT : (u + 1) * COUT] = conv_w[:, :, di, hj, wl + 1].T
            else:  # s34: upper half sees x shifted by +34 (next hj)
                wpk[64:128, u * COUT : (u + 1) * COUT] = conv_w[:, :, di, hj + 1, wl].T
    return wpk


def _win(t, parts, offset, dims):
    """Overlapping-window AP view of a [P, L] tile: free dims [(step, count), ...]."""
    v = t[0:parts, offset : offset + 1]
    for _ in range(len(dims) - 1):
        v = v.unsqueeze(1)
    w = v.copy()
    for i, (st, cnt) in enumerate(dims):
        w.ap[i + 1] = (st, cnt)
    return w


def _build_program_a():
    nc = bacc.Bacc(
        "TRN2", target_bir_lowering=False, debug=False, num_devices=N_CORES
    )
    xp2 = nc.dram_tensor("xp2", (128, XBUF), F32, kind="ExternalInput").ap()
    xq2 = nc.dram_tensor("xq2", (128, XBUF), F32, kind="ExternalInput").ap()
    wpk = nc.dram_tensor("wpk", (128, NUNITS * COUT), F32, kind="ExternalInput").ap()
    st = nc.dram_tensor("st", (COUT, 2), F32, kind="ExternalOutput").ap()
    ssub = nc.dram_tensor("ssub", (COUT, SPOS), F32, kind="ExternalOutput").ap()

    AX = mybir.AxisListType
    OP = mybir.AluOpType
    ACT = mybir.ActivationFunctionType

    with tile.TileContext(nc) as tc:
        with (
            tc.tile_pool(name="xin", bufs=1) as xin_pool,
            tc.tile_pool(name="consts", bufs=1) as const_pool,
            tc.tile_pool(name="stats", bufs=1) as stats_pool,
            tc.tile_pool(name="sq", bufs=2) as sq_pool,
        ):
            XPR = xin_pool.tile([128, XBUF], F32R)
            XS34 = xin_pool.tile([128, XBUF], F32R)
            WPK = const_pool.tile([128, NUNITS * COUT], F32)
            WPKR = const_pool.tile([128, NUNITS * COUT], F32R)

            nc.sync.dma_start(WPK[:], wpk[:])
            nc.vector.tensor_copy(WPKR[:], WPK[:])
            # input DMAs: stage per-plane, round f32 -> f32r into the slabs
            # (xp2: upper half +1-shifted; xq2: upper half +34-shifted)
            for z in range(ZPLANES):
                lo = z * PLANE
                hi = XBUF if z == ZPLANES - 1 else (z + 1) * PLANE
                STG = sq_pool.tile([128, PLANE + 8], F32, tag="stg")
                nc.sync.dma_start(STG[:, 0 : hi - lo], xp2[:, lo:hi])
                nc.vector.tensor_copy(XPR[:, lo:hi], STG[:, 0 : hi - lo])
                STG2 = sq_pool.tile([128, PLANE + 8], F32, tag="stg2")
                nc.scalar.dma_start(STG2[:, 0 : hi - lo], xq2[:, lo:hi])
                nc.vector.tensor_copy(XS34[:, lo:hi], STG2[:, 0 : hi - lo])

            SUMS = stats_pool.tile([COUT, DL], F32)
            SUMSQ = stats_pool.tile([COUT, DL], F32)
            SSUB = stats_pool.tile([COUT, SPOS], F32)

            # conv: per depth plane, flat span h*34+w (h<32, w<34; 2 junk
            # cols/row discarded at the stats/extraction step) in 3 groups
            SPAN = (PH - 2) * PW  # 1088
            GRPS = [(0, 364), (364, 364), (728, 360)]
            with tc.tile_pool(name="psum_conv", bufs=4, space="PSUM") as pconv:
                for d in range(DL):
                    SIG = sq_pool.tile([COUT, SPAN], F32, tag="sig")
                    for j0, ns in GRPS:
                        P = pconv.tile([COUT, 384], F32, tag="convps")
                        for u, (tn, di, hj, wl, ku) in enumerate(UNITS):
                            lhsT = WPKR[0:ku, u * COUT : (u + 1) * COUT]
                            base = (d + di) * PLANE + hj * PW + wl + j0
                            xt = XPR if tn == "s1" else XS34
                            rhs = xt[0:ku, base : base + ns]
                            nc.tensor.matmul(
                                P[:, 0:ns], lhsT, rhs,
                                start=(u == 0), stop=(u == NUNITS - 1),
                            )
                        nc.scalar.copy(SIG[:, j0 : j0 + ns], P[:, 0:ns])
                    # stats over the 32x32 valid positions of this plane
                    SIGv = SIG.rearrange("c (h w) -> c h w", h=H, w=PW)[:, :, 0:W]
                    nc.vector.tensor_reduce(
                        SUMS[:, d : d + 1], SIGv, axis=AX.XY, op=OP.add
                    )
                    SQT = sq_pool.tile([COUT, H * W], F32, tag="sqt")
                    SQTv = SQT.rearrange("c (h w) -> c h w", h=H, w=W)
                    nc.scalar.activation(
                        SQTv, SIGv, ACT.Square, accum_out=SUMSQ[:, d : d + 1]
                    )
                    if d % 2 == 0:
                        nc.scalar.copy(
                            SSUB[:, (d // 2) * 256 : (d // 2) * 256 + 256],
                            SIGv[:, 0:H:2, 0:W:2],
                        )

            ST = stats_pool.tile([COUT, 2], F32)
            nc.vector.tensor_reduce(ST[:, 0:1], SUMS[:], axis=AX.X, op=OP.add)
            nc.vector.tensor_reduce(ST[:, 1:2], SUMSQ[:], axis=AX.X, op=OP.add)
            nc.sync.dma_start(st[:], ST[:])
            nc.sync.dma_start(ssub[:], SSUB[:])
    nc.compile()
    return nc


def _build_program_b():
    nc = bacc.Bacc(
        "TRN2", target_bir_lowering=False, debug=False, num_devices=N_CORES
    )
    xpb = nc.dram_tensor("xpb", (64, XBUF), F16, kind="ExternalInput").ap()
    en = nc.dram_tensor("en", (4, 27 * GROUP * 256), F16, kind="ExternalInput").ap()
    out = nc.dram_tensor("out", (64, SPOS), F32, kind="ExternalOutput").ap()

    OP = mybir.AluOpType
    CH = 256  # one strided depth plane per chunk

    with tile.TileContext(nc) as tc:
        with (
            tc.tile_pool(name="xin", bufs=1) as xin_pool,
            tc.tile_pool(name="work", bufs=1) as work_pool,
        ):
            XPB = xin_pool.tile([64, XBUF], F16)
            # attention replication on the scalar HWDGE ring, slab loads on
            # the sync ring -> the two DMA streams don't queue behind each
            # other (HWDGE rings are FIFO per issuing engine)
            AREPS = [
                work_pool.tile([64, 27 * CH], F16, tag=f"arep{k}", name=f"AREP{k}")
                for k in range(4)
            ]
            # chunk 0's attention first, then slab planes interleaved with
            # the remaining attention blocks
            bounds = [0, 3 * PLANE, 5 * PLANE, 7 * PLANE, XBUF]
            for grp in range(2):
                nc.scalar.dma_start(
                    AREPS[0][grp * 32 : (grp + 1) * 32, :],
                    en[0:1, grp * 27 * CH : (grp + 1) * 27 * CH].partition_broadcast(32),
                )
            for k in range(1, 5):
                nc.sync.dma_start(
                    XPB[:, bounds[k - 1] : bounds[k]],
                    xpb[:, bounds[k - 1] : bounds[k]],
                )
                if k < 4:
                    for grp in range(2):
                        nc.scalar.dma_start(
                            AREPS[k][grp * 32 : (grp + 1) * 32, :],
                            en[
                                k : k + 1, grp * 27 * CH : (grp + 1) * 27 * CH
                            ].partition_broadcast(32),
                        )
            OUTC = work_pool.tile([64, SPOS], F32)
            for k in range(4):  # chunk = strided depth plane d4 = k
                AREP = AREPS[k]
                PRD = work_pool.tile([64, 27 * CH], F16, tag="prd", bufs=1)
                for di in range(K):
                    eng = nc.vector
                    for hj in range(K):
                        # patches [c, wl, h, w] = XPB[c, base + hj*34 + wl + h*68 + w*2]
                        xv = _win(
                            XPB, 64, (2 * k + di) * PLANE + hj * PW,
                            [(1, 3), (2 * PW, 16), (2, 16)],
                        )
                        sl = slice(
                            (di * 9 + hj * 3) * CH, (di * 9 + hj * 3 + 3) * CH
                        )
                        av = AREP[:, sl].rearrange(
                            "c (wl h w) -> c wl h w", wl=3, h=16, w=16
                        )
                        pvd = PRD[:, sl].rearrange(
                            "c (wl h w) -> c wl h w", wl=3, h=16, w=16
                        )
                        eng.tensor_tensor(pvd, xv, av, op=OP.mult)
                # reduce the 27 taps: halving tree on the (p, o) layout
                for lo, hi in [(11, 16), (8, 8), (4, 4), (2, 2)]:
                    nc.vector.tensor_add(
                        PRD[:, 0 : lo * CH],
                        PRD[:, 0 : lo * CH],
                        PRD[:, hi * CH : (hi + lo) * CH],
                    )
                nc.vector.tensor_add(
                    OUTC[:, k * CH : (k + 1) * CH],
                    PRD[:, 0:CH],
                    PRD[:, CH : 2 * CH],
                )
                nc.sync.dma_start(
                    out[:, k * CH : (k + 1) * CH], OUTC[:, k * CH : (k + 1) * CH]
                )
    nc.compile()
    return nc


def _prep_inputs_a(x, conv_w):
    xpad = np.pad(
        np.asarray(x, dtype=np.float32),
        ((0, 0), (0, 0), (1, 1), (1, 1), (1, 1)),
        mode="reflect",
    )
    wpk = _build_weight_pack(np.asarray(conv_w, dtype=np.float32))
    in_maps = []
    slabs = []
    for core in range(N_CORES):
        n, dc = core // 4, core % 4
        slab = xpad[n, :, 8 * dc : 8 * dc + ZPLANES].reshape(C, XLEN)
        xp2 = np.zeros((128, XBUF), dtype=np.float32)
        xp2[0:64, :XLEN] = slab
        xp2[64:128, : XLEN - 1] = slab[:, 1:]
        xq2 = np.zeros((128, XBUF), dtype=np.float32)
        xq2[0:64, :XLEN] = slab
        xq2[64:128, : XLEN - 34] = slab[:, 34:]
        in_maps.append({"xp2": xp2, "xq2": xq2, "wpk": wpk})
        slabs.append(np.ascontiguousarray(xp2[0:64]).astype(np.float16))
    return in_maps, slabs


def kernel(x, conv_w, bn_gamma, bn_beta):
    if "a" not in _PROGRAM_CACHE:
        _PROGRAM_CACHE["a"] = _build_program_a()
        _PROGRAM_CACHE["b"] = _build_program_b()
    nca, ncb = _PROGRAM_CACHE["a"], _PROGRAM_CACHE["b"]

    in_a, slabs = _prep_inputs_a(x, conv_w)
    res_a = bass_utils.run_bass_kernel_spmd(nca, in_a, core_ids=list(range(N_CORES)))

    # host: global BN stats (432 bytes per core), then normalized attention
    st = np.sum([r["st"] for r in res_a.results], axis=0, dtype=np.float64)
    mean = st[:, 0] / M_TOTAL
    var = st[:, 1] / M_TOTAL - mean * mean
    rstd = 1.0 / np.sqrt(var + EPS)
    a = np.asarray(bn_gamma, np.float64) * rstd
    b = np.asarray(bn_beta, np.float64) - mean * a

    in_b = []
    for core in range(N_CORES):
        ssub = res_a.results[core]["ssub"].astype(np.float64)
        e = np.exp(a[:, None] * ssub + b[:, None])
        en = (e / e.sum(axis=0, keepdims=True)).astype(np.float32)
        enr = np.ascontiguousarray(
            en.reshape(COUT, 4, 256).transpose(1, 0, 2)
        ).reshape(4, COUT * 256).astype(np.float16)
        in_b.append({"xpb": slabs[core], "en": enr})
    res_b = bass_utils.run_bass_kernel_spmd(ncb, in_b, core_ids=list(range(N_CORES)))

    full = np.empty((N, C, D // 2, H // 2, W // 2), dtype=np.float32)
    for core in range(N_CORES):
        n, dc = core // 4, core % 4
        full[n, :, 4 * dc : 4 * dc + 4] = res_b.results[core]["out"].reshape(
            64, 4, 16, 16
        )
    return full


# revision 36
# speedup vs baseline: 1.1094x; 1.0546x over previous
"""PASA group-softmax downsample kernel for 8 Trainium2 NeuronCores.

Reference computation (per reference.py):
  x (2, 64, 32, 32, 32) f32
  xp = reflect-pad x by 1 on d/h/w
  sigma = conv3d(xp, conv_w (54, 64, 3,3,3), stride 1, valid)   -> (2, 54, 32,32,32)
  sigma = batchnorm(sigma, batch stats over (n,d,h,w), gamma, beta)
  sigma = softmax(sigma, axis=1)
  out[n,g,cc,o] = sum_p patches[n,g,cc,p,o] * sigma[n,g*27+p,o]  (g=2 groups of 32 ch)
  return out[:, :, ::2, ::2, ::2]                                -> (2, 64, 16, 16, 16)

Sharding: 8 shards = (batch n in {0,1}) x (4 depth chunks of 8 planes).
Each core gets a padded depth slab with halo (10 planes of the padded
volume).

Two SPMD launches (a cross-core AllReduce measured ~80us +-70us of
launch-skew in this environment, vs a ~15us bare-launch floor, so the
432-byte BN-stat reduction is done on the host between launches):

Launch A (per core): stride-1 conv of the local slab as 27 shifted
  matmuls accumulated in PSUM (wl=0/1 tap pairs fused to K=128 via a
  +1-shifted copy of the slab in partitions 64..127; float32r at
  N>=256 runs 1 cycle/row). Per-plane BN partial sums + sum-of-squares
  and the strided-position sigma are extracted on DVE/ACT. Outputs:
  st (54, 2) partial stats, ssub (54, 1024) strided conv values.

Host: sum stats over cores -> mean/var -> a, b; E = exp(a*ssub + b);
  en = E / colsum(E)  (the group-softmax attention, pre-normalized).

Launch B (per core): for each strided depth plane (256 positions),
  replicate en rows across the 32 channels of each group with a
  partition-broadcast DMA, multiply against overlapping-window views
  of the fp32 slab (9 DVE multiplies of [64, 3x16x16] per chunk,
  products stored fp16 for the 2x 16-bit DVE rate), reduce the 27 taps
  with a contiguous halving tree in fp16 (final level re-expands to
  fp32), and DMA out (64, 4, 16, 16).
"""

import sys

sys.path.insert(0, "/opt/trn_rl_repo")

import numpy as np

import concourse.bacc as bacc
import concourse.mybir as mybir
from concourse import bass_utils, tile

N_CORES = 8
K = 3
GROUP = 2
STRIDE = 2
EPS = 1e-5

N, C, D, H, W = 2, 64, 32, 32, 32
COUT = GROUP * K * K * K  # 54
PD, PH, PW = D + 2, H + 2, W + 2  # 34, 34, 34
ZPLANES = 10  # 8 output planes + 2 halo planes of the padded volume
PLANE = PH * PW  # 1156
XLEN = ZPLANES * PLANE  # 11560
XBUF = XLEN + 8  # pad: junk-column reads of the last plane overrun slightly
DL = 8  # local output depth extent (stride-1)
SPOS = (DL // 2) * (H // 2) * (W // 2)  # 1024 strided positions per core
M_TOTAL = float(N * D * H * W)  # 65536 positions for BN stats

F32 = mybir.dt.float32
F32R = mybir.dt.float32r
F16 = mybir.dt.float16

# tap index p = di*9 + hj*3 + wl (matches reference im2col ordering)
# conv matmul units (tensor, di, hj, wl, K): 9 w-pairs (wl=0 with wl=1 via
# the +1-shifted upper half), 3 hj-pairs for wl=2 (hj=0 with hj=1 via a
# +34-shifted upper half), 3 singles (di, 2, 2) at K=64.
UNITS = (
    [("s1", di, hj, 0, 128) for di in range(K) for hj in range(K)]
    + [("s34", di, 0, 2, 128) for di in range(K)]
    + [("s1", di, 2, 2, 64) for di in range(K)]
)
NUNITS = len(UNITS)  # 15

_PROGRAM_CACHE = {}


def _build_weight_pack(conv_w: np.ndarray) -> np.ndarray:
    """Pack conv_w (54, 64, 3, 3, 3) into lhsT layout (128, 15*54)."""
    wpk = np.zeros((128, NUNITS * COUT), dtype=np.float32)
    for u, (tn, di, hj, wl, ku) in enumerate(UNITS):
        # lhsT[k, m]: k = input channel (row), m = output channel
        wpk[0:64, u * COUT : (u + 1) * COUT] = conv_w[:, :, di, hj, wl].T
        if ku == 128:
            if tn == "s1":  # upper half sees x shifted by +1 (next wl)
                wpk[64:128, u * COUT : (u + 1) * COUT] = conv_w[:, :, di, hj, wl + 1].T
            else:  # s34: upper half sees x shifted by +34 (next hj)
                wpk[64:128, u * COUT : (u + 1) * COUT] = conv_w[:, :, di, hj + 1, wl].T
    return wpk


def _win(t, parts, offset, dims):
    """Overlapping-window AP view of a [P, L] tile: free dims [(step, count), ...]."""
    v = t[0:parts, offset : offset + 1]
    for _ in range(len(dims) - 1):
        v = v.unsqueeze(1)
    w = v.copy()
    for i, (st, cnt) in enumerate(dims):
        w.ap[i + 1] = (st, cnt)
    return w


def _build_program_a():
    nc = bacc.Bacc(
        "TRN2", target_bir_lowering=False, debug=False, num_devices=N_CORES
    )
    xp2 = nc.dram_tensor("xp2", (128, XBUF), F32, kind="ExternalInput").ap()
    xq2 = nc.dram_tensor("xq2", (128, XBUF), F32, kind="ExternalInput").ap()
    wpk = nc.dram_tensor("wpk", (128, NUNITS * COUT), F32, kind="ExternalInput").ap()
    st = nc.dram_tensor("st", (COUT, 2), F32, kind="ExternalOutput").ap()
    ssub = nc.dram_tensor("ssub", (COUT, SPOS), F32, kind="ExternalOutput").ap()

    AX = mybir.AxisListType
    OP = mybir.AluOpType
    ACT = mybir.ActivationFunctionType

    with tile.TileContext(nc) as tc:
        with (
            tc.tile_pool(name="xin", bufs=1) as xin_pool,
            tc.tile_pool(name="consts", bufs=1) as const_pool,
            tc.tile_pool(name="stats", bufs=1) as stats_pool,
            tc.tile_pool(name="sq", bufs=2) as sq_pool,
        ):
            XPR = xin_pool.tile([128, XBUF], F32R)
            XS34 = xin_pool.tile([128, XBUF], F32R)
            WPK = const_pool.tile([128, NUNITS * COUT], F32)
            WPKR = const_pool.tile([128, NUNITS * COUT], F32R)

            nc.sync.dma_start(WPK[:], wpk[:])
            nc.vector.tensor_copy(WPKR[:], WPK[:])
            # input DMAs: stage per-plane, round f32 -> f32r into the slabs
            # (xp2: upper half +1-shifted; xq2: upper half +34-shifted)
            for z in range(ZPLANES):
                lo = z * PLANE
                hi = XBUF if z == ZPLANES - 1 else (z + 1) * PLANE
                STG = sq_pool.tile([128, PLANE + 8], F32, tag="stg")
                nc.sync.dma_start(STG[:, 0 : hi - lo], xp2[:, lo:hi])
                nc.vector.tensor_copy(XPR[:, lo:hi], STG[:, 0 : hi - lo])
                STG2 = sq_pool.tile([128, PLANE + 8], F32, tag="stg2")
                nc.scalar.dma_start(STG2[:, 0 : hi - lo], xq2[:, lo:hi])
                nc.vector.tensor_copy(XS34[:, lo:hi], STG2[:, 0 : hi - lo])

            SUMS = stats_pool.tile([COUT, DL], F32)
            SUMSQ = stats_pool.tile([COUT, DL], F32)
            SSUB = stats_pool.tile([COUT, SPOS], F32)

            # conv: per depth plane, flat span h*34+w (h<32, w<34; 2 junk
            # cols/row discarded at the stats/extraction step) in 3 groups
            SPAN = (PH - 2) * PW  # 1088
            GRPS = [(0, 364), (364, 364), (728, 360)]
            with tc.tile_pool(name="psum_conv", bufs=4, space="PSUM") as pconv:
                for d in range(DL):
                    SIG = sq_pool.tile([COUT, SPAN], F32, tag="sig")
                    for j0, ns in GRPS:
                        P = pconv.tile([COUT, 384], F32, tag="convps")
                        for u, (tn, di, hj, wl, ku) in enumerate(UNITS):
                            lhsT = WPKR[0:ku, u * COUT : (u + 1) * COUT]
                            base = (d + di) * PLANE + hj * PW + wl + j0
                            xt = XPR if tn == "s1" else XS34
                            rhs = xt[0:ku, base : base + ns]
                            nc.tensor.matmul(
                                P[:, 0:ns], lhsT, rhs,
                                start=(u == 0), stop=(u == NUNITS - 1),
                            )
                        nc.scalar.copy(SIG[:, j0 : j0 + ns], P[:, 0:ns])
                    # stats over the 32x32 valid positions of this plane
                    SIGv = SIG.rearrange("c (h w) -> c h w", h=H, w=PW)[:, :, 0:W]
                    nc.vector.tensor_reduce(
                        SUMS[:, d : d + 1], SIGv, axis=AX.XY, op=OP.add
                    )
                    SQT = sq_pool.tile([COUT, H * W], F32, tag="sqt")
                    SQTv = SQT.rearrange("c (h w) -> c h w", h=H, w=W)
                    nc.scalar.activation(
                        SQTv, SIGv, ACT.Square, accum_out=SUMSQ[:, d : d + 1]
                    )
                    if d % 2 == 0:
                        nc.scalar.copy(
                            SSUB[:, (d // 2) * 256 : (d // 2) * 256 + 256],
                            SIGv[:, 0:H:2, 0:W:2],
                        )

            ST = stats_pool.tile([COUT, 2], F32)
            nc.vector.tensor_reduce(ST[:, 0:1], SUMS[:], axis=AX.X, op=OP.add)
            nc.vector.tensor_reduce(ST[:, 1:2], SUMSQ[:], axis=AX.X, op=OP.add)
            nc.sync.dma_start(st[:], ST[:])
            nc.sync.dma_start(ssub[:], SSUB[:])
    nc.compile()
    return nc


def _build_program_b():
    nc = bacc.Bacc(
        "TRN2", target_bir_lowering=False, debug=False, num_devices=N_CORES
    )
    xpb = nc.dram_tensor("xpb", (64, ZPLANES * 4 * 289), F16, kind="ExternalInput").ap()
    en = nc.dram_tensor("en", (4, 27 * GROUP * 256), F16, kind="ExternalInput").ap()
    out = nc.dram_tensor("out", (64, SPOS), F32, kind="ExternalOutput").ap()

    OP = mybir.AluOpType
    CH = 256  # one strided depth plane per chunk

    with tile.TileContext(nc) as tc:
        with (
            tc.tile_pool(name="xin", bufs=1) as xin_pool,
            tc.tile_pool(name="work", bufs=1) as work_pool,
        ):
            XPB = xin_pool.tile([64, ZPLANES * 4 * 289], F16)
            # attention replication on the scalar HWDGE ring, slab loads on
            # the sync ring -> the two DMA streams don't queue behind each
            # other (HWDGE rings are FIFO per issuing engine)
            AREPS = [
                work_pool.tile([64, 27 * CH], F16, tag=f"arep{k}", name=f"AREP{k}")
                for k in range(4)
            ]
            # chunk 0's attention first, then slab planes interleaved with
            # the remaining attention blocks
            QP = 4 * 289  # one parity-decomposed plane
            bounds = [0, 3 * QP, 5 * QP, 7 * QP, ZPLANES * QP]
            for grp in range(2):
                nc.scalar.dma_start(
                    AREPS[0][grp * 32 : (grp + 1) * 32, :],
                    en[0:1, grp * 27 * CH : (grp + 1) * 27 * CH].partition_broadcast(32),
                )
            for k in range(1, 5):
                nc.sync.dma_start(
                    XPB[:, bounds[k - 1] : bounds[k]],
                    xpb[:, bounds[k - 1] : bounds[k]],
                )
                if k < 4:
                    for grp in range(2):
                        nc.scalar.dma_start(
                            AREPS[k][grp * 32 : (grp + 1) * 32, :],
                            en[
                                k : k + 1, grp * 27 * CH : (grp + 1) * 27 * CH
                            ].partition_broadcast(32),
                        )
            OUTC = work_pool.tile([64, SPOS], F32)
            for k in range(4):  # chunk = strided depth plane d4 = k
                AREP = AREPS[k]
                PRD = work_pool.tile([64, 27 * CH], F16, tag="prd", bufs=1)
                for di in range(K):
                    for hj in range(K):
                        # parity volume: xpp[c, z, hj%2, wl%2, h+hj//2, w+wl//2]
                        # (runs of 16 contiguous, pitch 17 -> 16-bit 2x mode)
                        zb = (2 * k + di) * QP + (hj % 2) * 2 * 289
                        row = (hj // 2) * 17
                        p0 = di * 9 + hj * 3
                        # wl in {0, 2}: same x-parity, col offsets 0 and 1
                        xv = _win(
                            XPB, 64, zb + row,
                            [(1, 2), (17, 16), (1, 16)],
                        )
                        av = _win(AREP, 64, p0 * CH, [(2 * CH, 2), (16, 16), (1, 16)])
                        pvd = _win(PRD, 64, p0 * CH, [(2 * CH, 2), (16, 16), (1, 16)])
                        nc.vector.tensor_tensor(pvd, xv, av, op=OP.mult)
                        # wl = 1: odd x-parity volume
                        xv1 = _win(
                            XPB, 64, zb + 289 + row,
                            [(17, 16), (1, 16)],
                        )
                        av1 = _win(AREP, 64, (p0 + 1) * CH, [(16, 16), (1, 16)])
                        pv1 = _win(PRD, 64, (p0 + 1) * CH, [(16, 16), (1, 16)])
                        nc.vector.tensor_tensor(pv1, xv1, av1, op=OP.mult)
                # reduce the 27 taps: halving tree on the (p, o) layout
                for lo, hi in [(11, 16), (8, 8), (4, 4), (2, 2)]:
                    nc.vector.tensor_add(
                        PRD[:, 0 : lo * CH],
                        PRD[:, 0 : lo * CH],
                        PRD[:, hi * CH : (hi + lo) * CH],
                    )
                nc.vector.tensor_add(
                    OUTC[:, k * CH : (k + 1) * CH],
                    PRD[:, 0:CH],
                    PRD[:, CH : 2 * CH],
                )
                nc.sync.dma_start(
                    out[:, k * CH : (k + 1) * CH], OUTC[:, k * CH : (k + 1) * CH]
                )
    nc.compile()
    return nc


def _prep_inputs_a(x, conv_w):
    xpad = np.pad(
        np.asarray(x, dtype=np.float32),
        ((0, 0), (0, 0), (1, 1), (1, 1), (1, 1)),
        mode="reflect",
    )
    wpk = _build_weight_pack(np.asarray(conv_w, dtype=np.float32))
    in_maps = []
    slabs = []
    for core in range(N_CORES):
        n, dc = core // 4, core % 4
        slab = xpad[n, :, 8 * dc : 8 * dc + ZPLANES].reshape(C, XLEN)
        xp2 = np.zeros((128, XBUF), dtype=np.float32)
        xp2[0:64, :XLEN] = slab
        xp2[64:128, : XLEN - 1] = slab[:, 1:]
        xq2 = np.zeros((128, XBUF), dtype=np.float32)
        xq2[0:64, :XLEN] = slab
        xq2[64:128, : XLEN - 34] = slab[:, 34:]
        in_maps.append({"xp2": xp2, "xq2": xq2, "wpk": wpk})
        s4 = xp2[0:64, :XLEN].reshape(C, ZPLANES, PH, PW)
        xpp = np.zeros((C, ZPLANES, 2, 2, 17, 17), dtype=np.float16)
        for py in range(2):
            for px in range(2):
                xpp[:, :, py, px] = s4[:, :, py::2, px::2]
        slabs.append(xpp.reshape(C, ZPLANES * 4 * 289))
    return in_maps, slabs


def kernel(x, conv_w, bn_gamma, bn_beta):
    if "a" not in _PROGRAM_CACHE:
        _PROGRAM_CACHE["a"] = _build_program_a()
        _PROGRAM_CACHE["b"] = _build_program_b()
    nca, ncb = _PROGRAM_CACHE["a"], _PROGRAM_CACHE["b"]

    in_a, slabs = _prep_inputs_a(x, conv_w)
    res_a = bass_utils.run_bass_kernel_spmd(nca, in_a, core_ids=list(range(N_CORES)))

    # host: global BN stats (432 bytes per core), then normalized attention
    st = np.sum([r["st"] for r in res_a.results], axis=0, dtype=np.float64)
    mean = st[:, 0] / M_TOTAL
    var = st[:, 1] / M_TOTAL - mean * mean
    rstd = 1.0 / np.sqrt(var + EPS)
    a = np.asarray(bn_gamma, np.float64) * rstd
    b = np.asarray(bn_beta, np.float64) - mean * a

    in_b = []
    for core in range(N_CORES):
        ssub = res_a.results[core]["ssub"].astype(np.float64)
        e = np.exp(a[:, None] * ssub + b[:, None])
        en = (e / e.sum(axis=0, keepdims=True)).astype(np.float32)
        enr = np.ascontiguousarray(
            en.reshape(COUT, 4, 256).transpose(1, 0, 2)
        ).reshape(4, COUT * 256).astype(np.float16)
        in_b.append({"xpb": slabs[core], "en": enr})
    res_b = bass_utils.run_bass_kernel_spmd(ncb, in_b, core_ids=list(range(N_CORES)))

    full = np.empty((N, C, D // 2, H // 2, W // 2), dtype=np.float32)
    for core in range(N_CORES):
        n, dc = core // 4, core % 4
        full[n, :, 4 * dc : 4 * dc + 4] = res_b.results[core]["out"].reshape(
            64, 4, 16, 16
        )
    return full
